# revision 1
# baseline (speedup 1.0000x reference)
"""LocalFrameAttentionWithDiffuser on 8 TRN2 NeuronCores.

Sharding: head-parallel. Each core computes 2 of the 16 heads end-to-end
(QKV projection for its 128 hd-dims, chunked local attention, partial
output projection Y_c = O_c @ Wo[c-slice]); the host sums the 8 partial
Y tensors (bias bo is fed only to core 0 so the sum adds it once).

Shapes (hardcoded from the problem):
  x [1,16,256,1024] -> tokens T=4096, D=1024, H=16 heads, HD=64,
  chunks C=4 of L=1024 tokens; chunk i attends to chunks {i-1, i}
  (chunk 0 only to itself).

Device layout notes:
  - everything flows transposed: X^T [D, T] is a host-prepared input so
    projections produce Q^T/K^T [hd, T] directly (hd on partitions).
  - S^T = K^T.T @ Q^T per (chunk, head) with ctx on partitions, so the
    softmax sum over ctx is computed by appending a ones-column to V in
    the AV matmul (row 64 of the AV PSUM accumulates sum(exp(s))).
  - chunk 0's missing previous chunk is handled by simply not issuing
    those ctx tiles (exactly reproduces the -inf mask).
  - matmuls use float32r (full-rate fp32 path on the PE).
"""

import os
from contextlib import ExitStack

import numpy as np

import concourse.bass as bass
import concourse.tile as tile
from concourse import bacc, mybir
from concourse.bass_utils import run_bass_kernel_spmd

F32 = mybir.dt.float32
F32R = mybir.dt.float32r

B, F, N, D = 1, 16, 256, 1024
H, HD = 16, 64
CS = 4
C = F // CS            # 4 chunks
L = CS * N             # 1024 tokens per chunk
T = F * N              # 4096 tokens
NCORES = 8
HPC = H // NCORES      # 2 heads per core
HDB = HPC * HD         # 128 hd dims per core
SCALE = 1.0 / np.sqrt(HD)

TOK_TILE = 512         # moving-dim tile (fp32 max)
NDT = D // 128         # 8 contraction tiles for projections
NJT = T // TOK_TILE    # 8 token tiles
NCT = T // 128         # 32 ctx tiles of 128


def _r(ap):
    return ap.bitcast(F32R)


def build_kernel(nc, tc, outs, ins, ctx, phases=3):
    xt, wq, wk, wv, wo, bo, ident = (
        ins["xt"], ins["wq"], ins["wk"], ins["wv"], ins["wo"], ins["bo"],
        ins["ident"],
    )
    y = outs["y"]

    # persistent pools: bufs=1, every tile gets a distinct name (= its own slot)
    wpool = ctx.enter_context(tc.tile_pool(name="weights", bufs=1))
    qk_pool = ctx.enter_context(tc.tile_pool(name="qk", bufs=1))
    v_pool = ctx.enter_context(tc.tile_pool(name="v", bufs=1))
    ot_pool = ctx.enter_context(tc.tile_pool(name="ot", bufs=1))
    ybias_pool = ctx.enter_context(tc.tile_pool(name="ybias", bufs=1))
    # cycling pools: shared tag -> bufs slots
    xpool = ctx.enter_context(tc.tile_pool(name="xt", bufs=16))
    vstage_pool = ctx.enter_context(tc.tile_pool(name="vstage", bufs=2))
    a_pool = ctx.enter_context(tc.tile_pool(name="attn", bufs=12))
    sum_pool = ctx.enter_context(tc.tile_pool(name="sums", bufs=8))
    bc_pool = ctx.enter_context(tc.tile_pool(name="bcast", bufs=4))
    yout_pool = ctx.enter_context(tc.tile_pool(name="yout", bufs=6))
    ps_pool = ctx.enter_context(tc.tile_pool(name="ps", bufs=8, space="PSUM"))
    proj_ps = vtr_ps = s_ps = o_ps = y_ps = ps_pool

    # ---- persistent weights / constants (loaded at first use) ----
    wq_sb = [wpool.tile([128, HDB], F32R, name=f"wq{d}") for d in range(NDT)]
    wk_sb = [wpool.tile([128, HDB], F32R, name=f"wk{d}") for d in range(NDT)]
    wv_sb = [wpool.tile([128, HDB], F32R, name=f"wv{d}") for d in range(NDT)]
    wo_sb = wpool.tile([128, D], F32R, tag="wo")
    id_sb = wpool.tile([128, 128], F32, tag="id")
    ones_col = wpool.tile([128, 1], F32, tag="ones")
    nc.vector.memset(ones_col[:], 1.0)
    bo_bc = ybias_pool.tile([128, D], F32)

    # persistent activations
    qt_sb = qk_pool.tile([128, T], F32R, tag="qt")   # Q^T (2 heads stacked)
    kt_sb = qk_pool.tile([128, T], F32R, tag="kt")   # K^T
    ot_sb = ot_pool.tile([128, T], F32R)             # O^T normalized
    # V per ctx tile: [128 tok, 65] (64 hd + ones column), per head
    v_sb = [[v_pool.tile([128, HD + 1], F32R, name=f"v{h}_{ct}") for ct in range(NCT)]
            for h in range(HPC)]

    # ---- phase 1: projections (per 512-token tile) ----
    for j in range(NJT):
        tok = bass.ts(j, TOK_TILE)
        xt_t = [xpool.tile([128, TOK_TILE], F32R, tag="x", name=f"xt{j}_{d}") for d in range(NDT)]
        for d in range(NDT):
            nc.sync.dma_start(xt_t[d][:], xt[d * 128:(d + 1) * 128, tok].bitcast(F32R))
            if j == 0:
                nc.sync.dma_start(wq_sb[d][:], wq[d * 128:(d + 1) * 128, :].bitcast(F32R))
                nc.sync.dma_start(wk_sb[d][:], wk[d * 128:(d + 1) * 128, :].bitcast(F32R))
                nc.sync.dma_start(wv_sb[d][:], wv[d * 128:(d + 1) * 128, :].bitcast(F32R))
        if j == 0:
            nc.sync.dma_start(id_sb[:], ident[:, :])
        if j == 2:
            nc.sync.dma_start(wo_sb[:], wo[:, :].bitcast(F32R))
            nc.sync.dma_start(bo_bc[:], bo[0:1, :].broadcast_to([128, D]))
        q_ps = proj_ps.tile([128, TOK_TILE], F32, tag="ps", name=f"qps{j}")
        k_ps = proj_ps.tile([128, TOK_TILE], F32, tag="ps", name=f"kps{j}")
        vt_ps = proj_ps.tile([128, TOK_TILE], F32, tag="ps", name=f"vps{j}")
        for d in range(NDT):
            st, sp = d == 0, d == NDT - 1
            nc.tensor.matmul(q_ps[:], wq_sb[d][:], xt_t[d][:], start=st, stop=sp)
            nc.tensor.matmul(k_ps[:], wk_sb[d][:], xt_t[d][:], start=st, stop=sp)
            nc.tensor.matmul(vt_ps[:], wv_sb[d][:], xt_t[d][:], start=st, stop=sp)
        nc.vector.tensor_copy(qt_sb[:, tok], q_ps[:])
        nc.vector.tensor_copy(kt_sb[:, tok], k_ps[:])
        vt_stage = vstage_pool.tile([128, TOK_TILE], F32, tag="vs", name=f"vst{j}")
        nc.vector.tensor_copy(vt_stage[:], vt_ps[:])
        # transpose V^T -> V in 128x128 blocks; split the two heads
        for kblk in range(TOK_TILE // 128):
            ct = j * (TOK_TILE // 128) + kblk
            vtr = vtr_ps.tile([128, 128], F32, tag="ps", name=f"vtr{j}_{kblk}")
            nc.tensor.transpose(vtr[:], vt_stage[:, bass.ts(kblk, 128)], id_sb[:])
            for h in range(HPC):
                nc.vector.tensor_copy(v_sb[h][ct][:, 0:HD], vtr[:, h * HD:(h + 1) * HD])
                nc.gpsimd.tensor_copy(v_sb[h][ct][:, HD:HD + 1], ones_col[:])

    # ---- phase 2+3: attention per chunk, then its slice of the output proj ----
    if phases < 2:
        return
    for c in range(C):
        cts = list(range(max(0, 8 * (c - 1)), 8 * (c + 1)))  # ctx tiles (128 tok)
        for th in range(L // TOK_TILE):  # 2 query halves per chunk
            tok0 = c * L + th * TOK_TILE
            tok = bass.ds(tok0, TOK_TILE)
            for h in range(HPC):
                hr = slice(h * HD, (h + 1) * HD)
                o_acc = o_ps.tile([HD + 1, TOK_TILE], F32, tag="ps", name=f"ops{c}_{th}_{h}")
                for ci, ct in enumerate(cts):
                    s_t = s_ps.tile([128, TOK_TILE], F32, tag="ps", name=f"sps{c}_{th}_{h}_{ci}")
                    nc.tensor.matmul(
                        s_t[:], kt_sb[hr, bass.ts(ct, 128)], qt_sb[hr, tok],
                        start=True, stop=True,
                    )
                    a_t = a_pool.tile([128, TOK_TILE], F32R, tag="a", name=f"a{c}_{th}_{h}_{ci}")
                    nc.scalar.activation(
                        a_t[:], s_t[:], mybir.ActivationFunctionType.Exp, scale=SCALE
                    )
                    nc.tensor.matmul(
                        o_acc[:], v_sb[h][ct][:], a_t[:],
                        start=(ci == 0), stop=(ci == len(cts) - 1),
                    )
                # normalize: rows 0:64 / row 64
                s_sum = sum_pool.tile([1, TOK_TILE], F32, tag="s", name=f"ssum{c}_{th}_{h}")
                nc.vector.reciprocal(s_sum[:], o_acc[HD:HD + 1, :])
                r_bc = bc_pool.tile([HD, TOK_TILE], F32, tag="bc", name=f"bc{c}_{th}_{h}")
                nc.gpsimd.partition_broadcast(r_bc[:], s_sum[0:1, :])
                nc.vector.tensor_mul(ot_sb[hr, tok], o_acc[0:HD, :], r_bc[:])
            # output projection for this half-chunk's 4 token tiles
            for m in ([] if phases < 3 else range(8 * c + 4 * th, 8 * c + 4 * (th + 1))):
                for dh in range(D // TOK_TILE):
                    yp = y_ps.tile([128, TOK_TILE], F32, tag="ps", name=f"yps{m}_{dh}")
                    nc.tensor.matmul(
                        yp[:], ot_sb[:, bass.ts(m, 128)],
                        wo_sb[:, bass.ts(dh, TOK_TILE)],
                        start=True, stop=True,
                    )
                    y_sb = yout_pool.tile([128, TOK_TILE], F32, tag="yo", name=f"yo{m}_{dh}")
                    nc.vector.tensor_add(y_sb[:], yp[:], bo_bc[:, bass.ts(dh, TOK_TILE)])
                    nc.sync.dma_start(y[bass.ts(m, 128), bass.ts(dh, TOK_TILE)], y_sb[:])


_CACHE = {}


def _build(phases=3):
    if ("nc", phases) in _CACHE:
        return _CACHE[("nc", phases)]
    nc = bacc.Bacc(
        "TRN2",
        target_bir_lowering=False,
        debug=False,
        enable_asserts=False,
        num_devices=NCORES,
    )
    ins = {
        "xt": nc.dram_tensor("xt", [D, T], F32, kind="ExternalInput").ap(),
        "wq": nc.dram_tensor("wq", [D, HDB], F32, kind="ExternalInput").ap(),
        "wk": nc.dram_tensor("wk", [D, HDB], F32, kind="ExternalInput").ap(),
        "wv": nc.dram_tensor("wv", [D, HDB], F32, kind="ExternalInput").ap(),
        "wo": nc.dram_tensor("wo", [HDB, D], F32, kind="ExternalInput").ap(),
        "bo": nc.dram_tensor("bo", [1, D], F32, kind="ExternalInput").ap(),
        "ident": nc.dram_tensor("ident", [128, 128], F32, kind="ExternalInput").ap(),
    }
    outs = {"y": nc.dram_tensor("y", [T, D], F32, kind="ExternalOutput").ap()}
    with tile.TileContext(nc, trace_sim=False) as tc:
        with ExitStack() as kctx:
            build_kernel(nc, tc, outs, ins, kctx, phases=phases)
    nc.compile()
    _CACHE[("nc", phases)] = nc
    return nc


def make_in_maps(x, Wq, Wk, Wv, Wo, bo):
    xt = np.ascontiguousarray(
        np.asarray(x, dtype=np.float32).reshape(T, D).T
    )
    ident = np.eye(128, dtype=np.float32)
    bo = np.asarray(bo, dtype=np.float32).reshape(1, D)
    zeros_bo = np.zeros_like(bo)
    in_maps = []
    for core in range(NCORES):
        hs = slice(core * HDB, (core + 1) * HDB)
        in_maps.append({
            "xt": xt,
            "wq": np.ascontiguousarray(np.asarray(Wq, np.float32)[:, hs]),
            "wk": np.ascontiguousarray(np.asarray(Wk, np.float32)[:, hs]),
            "wv": np.ascontiguousarray(np.asarray(Wv, np.float32)[:, hs]),
            "wo": np.ascontiguousarray(np.asarray(Wo, np.float32)[hs, :]),
            "bo": bo if core == 0 else zeros_bo,
            "ident": ident,
        })
    return in_maps


def kernel(x, Wq, Wk, Wv, Wo, bo, _trace=False, _tmpdir=None):
    nc = _build()
    in_maps = make_in_maps(x, Wq, Wk, Wv, Wo, bo)
    res = run_bass_kernel_spmd(
        nc, in_maps, core_ids=list(range(NCORES)),
        trace=_trace, tmpdir=_tmpdir,
        **({"trace_cores": list(range(NCORES))} if _trace else {}),
    )
    if _trace:
        kernel.last_results = res
    y = np.zeros((T, D), dtype=np.float32)
    for r in res.results:
        y += r["y"]
    return y.reshape(B, F, N, D)



# revision 11
# speedup vs baseline: 1.2730x; 1.2730x over previous
"""LocalFrameAttentionWithDiffuser on 8 TRN2 NeuronCores.

Sharding: head-parallel. Each core computes 2 of the 16 heads end-to-end
(QKV projection for its 128 hd-dims, chunked local attention, partial
output projection Y_c = O_c @ Wo[c-slice]); the host sums the 8 partial
Y tensors and adds the bias once.

Shapes (hardcoded from the problem):
  x [1,16,256,1024] -> tokens T=4096, D=1024, H=16 heads, HD=64,
  chunks C=4 of L=1024 tokens; chunk i attends to chunks {i-1, i}
  (chunk 0 only to itself).

v2 design notes (cost-model driven):
  - bf16 datapath everywhere on SBUF (PSUM accumulates fp32): matmul
    rate is unchanged vs float32r at moving>=256, but bf16 keeps the
    full 1 cyc/row rate at small moving sizes, halves DMA bytes and
    speeds DVE copies.
  - scores S^T [ctx, q] per (chunk, head): PSUM tile [128, 1024]
    holds TWO 128-ctx tiles for 512 queries; ONE exp activation
    covers both (fewer Act instructions - Act is the bottleneck engine
    at ~116us; every score element must pass through it).
  - AV is emitted "flipped": stationary = A tile [128 ctx, 128 q],
    moving = V' [128 ctx, 65] (64 hd dims + ones column -> softmax
    denominator lands in column 64). Moving size 65 instead of 512
    halves the PE cost of AV.
  - normalization is a per-partition reciprocal + tensor_scalar mul
    into [tok, hd] staging, then a PE transpose back to [hd, tok] for
    the output projection.
  - V is projected directly in [tok, hd] orientation (stationary =
    x^T tile, moving = Wv tile) - no V transposes.
  - single interleaved emission schedule: projection chains /
    transposes / output-projection tiles are spread between attention
    pair-events as PE filler so the PE never idles (the cost model
    halves PE speed for 3us after any idle gap).
  - y partial [T, D] fp32 is DMA'd straight from PSUM (no engine copy);
    x/weights ship as bf16; DMAs are coalesced (the HWDGE device is
    serial at ~630ns per dma_start).
"""

from contextlib import ExitStack

import numpy as np

import concourse.bass as bass
import concourse.tile as tile
from concourse import bacc, mybir
from concourse.bass_utils import run_bass_kernel_spmd

F32 = mybir.dt.float32
BF16 = mybir.dt.bfloat16

B, F, N, D = 1, 16, 256, 1024
H, HD = 16, 64
CS = 4
C = F // CS            # 4 chunks
L = CS * N             # 1024 tokens per chunk
T = F * N              # 4096 tokens
NCORES = 8
HPC = H // NCORES      # 2 heads per core
HDB = HPC * HD         # 128 hd dims per core
SCALE = 1.0 / np.sqrt(HD)

NDT = D // 128         # 8 contraction tiles for projections
NQT = T // 512         # 8 query-projection tiles (512 tokens each)
NCT = T // 128         # 32 ctx tiles of 128 tokens
NTB = T // 128         # 32 token tiles (output side)

# ctx tiles seen by chunk c (128-token tiles, global index)
CTS = [list(range(max(0, 8 * (c - 1)), 8 * (c + 1))) for c in range(C)]
PAIRS = [[(t[i], t[i + 1]) for i in range(0, len(t), 2)] for t in CTS]
# attention pair events: (c, th, h, p); th = 512-query half of the chunk
EVENTS = [
    (c, th, h, p)
    for c in range(C)
    for th in (0, 1)
    for h in range(HPC)
    for p in range(len(PAIRS[c]))
]
assert len(EVENTS) == 112

# static PE filler schedule: event index -> list of (kind, idx)
# kinds: Q/K = 512-token q/k projection chain j, V = 128-token v
# projection ct, P = post work for token tile tb (transpose + out-proj)
FILLER = {
    0: [("V", 0), ("V", 1), ("V", 2)],
    1: [("K", 1), ("V", 3)],
    2: [("V", 4), ("V", 5)],
    3: [("V", 6), ("V", 7)],
    4: [("Q", 1)],
    5: [("K", 2)],
    6: [("Q", 2)],
    7: [("K", 3)],
    8: [("Q", 3)],
    9: [("V", 8)], 10: [("V", 9)], 11: [("V", 10)], 12: [("V", 11)],
    16: [("V", 12)], 17: [("V", 13)], 18: [("V", 14)], 19: [("V", 15)],
    20: [("Q", 4)], 21: [("K", 4)], 22: [("Q", 5)], 23: [("K", 5)],
    24: [("P", 0)], 26: [("P", 1)], 28: [("P", 2)], 30: [("P", 3)],
    32: [("V", 16)], 33: [("V", 17)], 34: [("V", 18)], 35: [("V", 19)],
    36: [("P", 4)], 38: [("P", 5)], 40: [("P", 6)], 42: [("P", 7)],
    48: [("V", 20)], 49: [("V", 21)], 50: [("V", 22)], 51: [("V", 23)],
    52: [("Q", 6)], 53: [("K", 6)], 54: [("Q", 7)], 55: [("K", 7)],
    56: [("P", 8)], 58: [("P", 9)], 60: [("P", 10)], 62: [("P", 11)],
    64: [("V", 24)], 65: [("V", 25)], 66: [("V", 26)], 67: [("V", 27)],
    68: [("P", 12)], 70: [("P", 13)], 72: [("P", 14)], 74: [("P", 15)],
    80: [("V", 28)], 81: [("V", 29)], 82: [("V", 30)], 83: [("V", 31)],
    84: [("P", 16)], 86: [("P", 17)], 88: [("P", 18)], 90: [("P", 19)],
    92: [("P", 20)], 94: [("P", 21)], 96: [("P", 22)], 98: [("P", 23)],
    100: [("P", 24)], 102: [("P", 25)], 104: [("P", 26)], 106: [("P", 27)],
}


def build_kernel(nc, tc, outs, ins, ctx):
    xt, wq, wk, wv, wo, ident = (
        ins["xt"], ins["wq"], ins["wk"], ins["wv"], ins["wo"], ins["ident"],
    )
    y = outs["y"]

    # ---- SBUF pools ----
    wpool = ctx.enter_context(tc.tile_pool(name="weights", bufs=1))
    xpool = ctx.enter_context(tc.tile_pool(name="xtiles", bufs=1))
    qk_pool = ctx.enter_context(tc.tile_pool(name="qk", bufs=1))
    v_pool = ctx.enter_context(tc.tile_pool(name="vsb", bufs=1))
    a_pool = ctx.enter_context(tc.tile_pool(name="attn", bufs=4))
    osb_pool = ctx.enter_context(tc.tile_pool(name="osb", bufs=12))
    r_pool = ctx.enter_context(tc.tile_pool(name="recip", bufs=4))
    ot_pool = ctx.enter_context(tc.tile_pool(name="ot", bufs=1))
    ysb_pool = ctx.enter_context(tc.tile_pool(name="ysb", bufs=2))
    # PSUM: s 2x2 banks + o 1 + p 1 + y 2 = 8 banks exactly
    pp = ctx.enter_context(tc.tile_pool(name="pp", bufs=1, space="PSUM"))

    # persistent SBUF tiles
    wq_sb = wpool.tile([128, D], BF16, name="wqsb")
    wk_sb = wpool.tile([128, D], BF16, name="wksb")
    wv_sb = wpool.tile([128, D], BF16, name="wvsb")
    wo_sb = wpool.tile([128, D], BF16, name="wosb")
    id_sb = wpool.tile([128, 128], BF16, name="idsb")
    xt_t = [xpool.tile([128, T], BF16, name=f"xt{d}") for d in range(NDT)]
    qt_sb = qk_pool.tile([128, T], BF16, name="qtsb")   # [2 heads x 64, T]
    kt_sb = qk_pool.tile([128, T], BF16, name="ktsb")
    # V per head: [128 ctx-token partitions, 32 ctx tiles, 64 hd + ones col]
    v_sb = [v_pool.tile([128, NCT, HD + 1], BF16, name=f"vsb{h}")
            for h in range(HPC)]
    ot_sb = ot_pool.tile([128, T], BF16, name="otsb")   # O^T normalized

    # ---- input DMAs (HWDGE is serial: few, large transfers; ordered so
    # the first projection chains unblock as early as possible) ----
    nc.sync.dma_start(wq_sb[:], wq[:, :])
    nc.sync.dma_start(wk_sb[:], wk[:, :])
    for d in range(NDT):   # tokens 0..511 of every contraction tile
        nc.sync.dma_start(xt_t[d][:, 0:512], xt[d * 128:(d + 1) * 128, 0:512])
    nc.sync.dma_start(wv_sb[:], wv[:, :])
    nc.sync.dma_start(wo_sb[:], wo[:, :])
    nc.sync.dma_start(id_sb[:], ident[:, :])
    for d in range(NDT):   # tokens 512..1023
        nc.sync.dma_start(xt_t[d][:, 512:1024], xt[d * 128:(d + 1) * 128, 512:1024])
    for d in range(NDT):   # rest
        nc.sync.dma_start(xt_t[d][:, 1024:T], xt[d * 128:(d + 1) * 128, 1024:T])

    # ones columns of V (softmax denominator rides the AV matmul)
    for h in range(HPC):
        nc.vector.memset(v_sb[h][:, :, HD], 1.0)

    # ---- emission helpers ----
    def emit_Q(j, which):
        w_sb, dst = (wq_sb, qt_sb) if which == "Q" else (wk_sb, kt_sb)
        ps = pp.tile([128, 512], F32, tag="p", bufs=1, name=f"{which}ps{j}")
        for d in range(NDT):
            nc.tensor.matmul(
                ps[:], w_sb[:, d * 128:(d + 1) * 128],
                xt_t[d][:, j * 512:(j + 1) * 512],
                start=(d == 0), stop=(d == NDT - 1),
            )
        nc.vector.tensor_copy(dst[:, j * 512:(j + 1) * 512], ps[:])

    def emit_V(ct):
        ps = pp.tile([128, 128], F32, tag="p", bufs=1, name=f"vps{ct}")
        for d in range(NDT):
            nc.tensor.matmul(
                ps[:], xt_t[d][:, ct * 128:(ct + 1) * 128],
                wv_sb[:, d * 128:(d + 1) * 128],
                start=(d == 0), stop=(d == NDT - 1),
            )
        for h in range(HPC):
            nc.vector.tensor_copy(
                v_sb[h][:, ct, 0:HD], ps[:, h * HD:(h + 1) * HD])

    osb_tiles = {}

    def get_osb(tb):
        if tb not in osb_tiles:
            osb_tiles[tb] = osb_pool.tile(
                [128, 128], BF16, tag="osb", name=f"osb{tb}")
        return osb_tiles[tb]

    ysb_tiles = {}

    def emit_post(tb):
        g = tb // 4
        if g not in ysb_tiles:
            ysb_tiles[g] = ysb_pool.tile(
                [128, 4, D], BF16, tag="ysb", name=f"ysb{g}")
        ysb = ysb_tiles[g]
        t_ps = pp.tile([128, 128], BF16, tag="p", bufs=1, name=f"tp{tb}")
        nc.tensor.transpose(t_ps[:], osb_tiles[tb][:], id_sb[:])
        nc.vector.tensor_copy(ot_sb[:, tb * 128:(tb + 1) * 128], t_ps[:])
        yp = pp.tile([128, D], F32, tag="y", bufs=1, name=f"yp{tb}")
        for dh in range(D // 512):
            nc.tensor.matmul(
                yp[:, dh * 512:(dh + 1) * 512],
                ot_sb[:, tb * 128:(tb + 1) * 128],
                wo_sb[:, dh * 512:(dh + 1) * 512],
                start=True, stop=True,
            )
        nc.vector.tensor_copy(ysb[:, tb % 4, :], yp[:])
        if tb % 4 == 3:
            # y is [8, 4, 128, 1024]; dst iterates [p][tb][d] to match the
            # SBUF staging tile's element order
            nc.sync.dma_start(
                y[g:g + 1].transpose([0, 2, 1, 3]), ysb[:])

    def emit_filler(kind, idx):
        if kind in ("Q", "K"):
            emit_Q(idx, kind)
        elif kind == "V":
            emit_V(idx)
        else:
            emit_post(idx)

    def emit_spair(e, c, th, h, p):
        ct0, ct1 = PAIRS[c][p]
        tok0 = c * L + th * 512
        hr = slice(h * HD, (h + 1) * HD)
        s_t = pp.tile([128, 1024], F32, tag="s", bufs=2, name=f"s{e}")
        nc.tensor.matmul(
            s_t[:, 0:512], kt_sb[hr, ct0 * 128:(ct0 + 1) * 128],
            qt_sb[hr, tok0:tok0 + 512], start=True, stop=True)
        nc.tensor.matmul(
            s_t[:, 512:1024], kt_sb[hr, ct1 * 128:(ct1 + 1) * 128],
            qt_sb[hr, tok0:tok0 + 512], start=True, stop=True)
        a_t = a_pool.tile([128, 1024], BF16, tag="a", name=f"a{e}")
        nc.scalar.activation(
            a_t[:], s_t[:], mybir.ActivationFunctionType.Exp, scale=SCALE)
        return a_t

    o_tiles = {}

    def emit_oinit(c, th, h):
        # matmul start=True zeroes the WHOLE psum bank, so the four
        # interleaved qb accumulation regions cannot each use start.
        # Zero the tile once on DVE and accumulate with start=False.
        o_t = pp.tile(
            [128, 4, HD + 1], F32, tag="o", bufs=1, name=f"o{c}_{th}_{h}")
        o_tiles[(c, th, h)] = o_t
        nc.vector.memset(o_t[:], 0.0)

    def emit_av(c, th, h, p, a_t):
        npair = len(PAIRS[c])
        o_t = o_tiles[(c, th, h)]
        for ci, ct in enumerate(PAIRS[c][p]):
            for qb in range(4):
                nc.tensor.matmul(
                    o_t[:, qb, :],
                    a_t[:, ci * 512 + qb * 128: ci * 512 + (qb + 1) * 128],
                    v_sb[h][:, ct, :],
                    start=False,
                    stop=(p == npair - 1 and ci == 1),
                    skip_group_check=True,
                )

    def emit_normalize(c, th, h):
        o_t = o_tiles[(c, th, h)]
        r = r_pool.tile([128, 4], F32, tag="r", name=f"r{c}_{th}_{h}")
        nc.vector.reciprocal(r[:], o_t[:, :, HD])
        for qb in range(4):
            tb = c * 8 + th * 4 + qb
            osb = get_osb(tb)
            nc.vector.tensor_scalar_mul(
                osb[:, h * HD:(h + 1) * HD], o_t[:, qb, 0:HD], r[:, qb:qb + 1])

    # ---- main interleaved emission ----
    emit_Q(0, "Q")
    emit_Q(0, "K")

    pending_av = None   # (c, th, h, p, a_t) awaiting emission (lag 1)
    prev_cthh = None
    for e, (c, th, h, p) in enumerate(EVENTS):
        if prev_cthh is not None and prev_cthh != (c, th, h):
            # drain previous (c, th, h): tail AV + normalization
            pc, pth, ph, pp_, pa = pending_av
            emit_av(pc, pth, ph, pp_, pa)
            pending_av = None
            emit_normalize(pc, pth, ph)
        if p == 0:
            emit_oinit(c, th, h)
        a_t = emit_spair(e, c, th, h, p)
        if pending_av is not None:
            emit_av(*pending_av)
        pending_av = (c, th, h, p, a_t)
        prev_cthh = (c, th, h)
        for kind, idx in FILLER.get(e, ()):
            emit_filler(kind, idx)

    # tail
    pc, pth, ph, pp_, pa = pending_av
    emit_av(pc, pth, ph, pp_, pa)
    emit_normalize(pc, pth, ph)
    for tb in range(28, 32):
        emit_post(tb)


_CACHE = {}


def _build():
    if "nc" in _CACHE:
        return _CACHE["nc"]
    nc = bacc.Bacc(
        "TRN2",
        target_bir_lowering=False,
        debug=False,
        enable_asserts=False,
        num_devices=NCORES,
    )
    ins = {
        "xt": nc.dram_tensor("xt", [D, T], BF16, kind="ExternalInput").ap(),
        "wq": nc.dram_tensor("wq", [128, D], BF16, kind="ExternalInput").ap(),
        "wk": nc.dram_tensor("wk", [128, D], BF16, kind="ExternalInput").ap(),
        "wv": nc.dram_tensor("wv", [128, D], BF16, kind="ExternalInput").ap(),
        "wo": nc.dram_tensor("wo", [128, D], BF16, kind="ExternalInput").ap(),
        "ident": nc.dram_tensor("ident", [128, 128], BF16, kind="ExternalInput").ap(),
    }
    outs = {"y": nc.dram_tensor(
        "y", [T // 512, 4, 128, D], BF16, kind="ExternalOutput").ap()}
    with tile.TileContext(nc, trace_sim=False) as tc:
        with ExitStack() as kctx:
            build_kernel(nc, tc, outs, ins, kctx)
    nc.compile()
    _CACHE["nc"] = nc
    return nc


def make_in_maps(x, Wq, Wk, Wv, Wo, bo):
    BF = mybir.dt.np(mybir.dt.bfloat16)
    xt = np.ascontiguousarray(
        np.asarray(x, dtype=np.float32).reshape(T, D).T
    ).astype(BF)
    ident = np.eye(128, dtype=np.float32).astype(BF)

    def pack(w):  # [1024, 128] -> [128, 8*128], ktile d at cols d*128
        return np.ascontiguousarray(np.concatenate(
            [w[d * 128:(d + 1) * 128, :] for d in range(NDT)], axis=1
        )).astype(BF)

    in_maps = []
    for core in range(NCORES):
        hs = slice(core * HDB, (core + 1) * HDB)
        in_maps.append({
            "xt": xt,
            "wq": pack(np.asarray(Wq, np.float32)[:, hs]),
            "wk": pack(np.asarray(Wk, np.float32)[:, hs]),
            "wv": pack(np.asarray(Wv, np.float32)[:, hs]),
            "wo": np.ascontiguousarray(
                np.asarray(Wo, np.float32)[hs, :]).astype(BF),
            "ident": ident,
        })
    return in_maps


def kernel(x, Wq, Wk, Wv, Wo, bo, _trace=False, _tmpdir=None):
    nc = _build()
    in_maps = make_in_maps(x, Wq, Wk, Wv, Wo, bo)
    res = run_bass_kernel_spmd(
        nc, in_maps, core_ids=list(range(NCORES)),
        trace=_trace, tmpdir=_tmpdir,
        **({"trace_cores": list(range(NCORES))} if _trace else {}),
    )
    if _trace:
        kernel.last_results = res
    y = np.zeros((T, D), dtype=np.float32)
    for r in res.results:
        y += np.asarray(r["y"], dtype=np.float32).reshape(T, D)
    y += np.asarray(bo, dtype=np.float32).reshape(1, D)
    return y.reshape(B, F, N, D)


# revision 13
# speedup vs baseline: 1.3155x; 1.0334x over previous
"""LocalFrameAttentionWithDiffuser on 8 TRN2 NeuronCores.

Sharding: head-parallel. Each core computes 2 of the 16 heads end-to-end
(QKV projection for its 128 hd-dims, chunked local attention, partial
output projection Y_c = O_c @ Wo[c-slice]); the host sums the 8 partial
Y tensors and adds the bias once.

Shapes (hardcoded from the problem):
  x [1,16,256,1024] -> tokens T=4096, D=1024, H=16 heads, HD=64,
  chunks C=4 of L=1024 tokens; chunk i attends to chunks {i-1, i}
  (chunk 0 only to itself).

v2 design notes (cost-model driven):
  - bf16 datapath everywhere on SBUF (PSUM accumulates fp32): matmul
    rate is unchanged vs float32r at moving>=256, but bf16 keeps the
    full 1 cyc/row rate at small moving sizes, halves DMA bytes and
    speeds DVE copies.
  - scores S^T [ctx, q] per (chunk, head): PSUM tile [128, 1024]
    holds TWO 128-ctx tiles for 512 queries; ONE exp activation
    covers both (fewer Act instructions - Act is the bottleneck engine
    at ~116us; every score element must pass through it).
  - AV is emitted "flipped": stationary = A tile [128 ctx, 128 q],
    moving = V' [128 ctx, 65] (64 hd dims + ones column -> softmax
    denominator lands in column 64). Moving size 65 instead of 512
    halves the PE cost of AV.
  - normalization is a per-partition reciprocal + tensor_scalar mul
    into [tok, hd] staging, then a PE transpose back to [hd, tok] for
    the output projection.
  - V is projected directly in [tok, hd] orientation (stationary =
    x^T tile, moving = Wv tile) - no V transposes.
  - single interleaved emission schedule: projection chains /
    transposes / output-projection tiles are spread between attention
    pair-events as PE filler so the PE never idles (the cost model
    halves PE speed for 3us after any idle gap).
  - y partial [T, D] fp32 is DMA'd straight from PSUM (no engine copy);
    x/weights ship as bf16; DMAs are coalesced (the HWDGE device is
    serial at ~630ns per dma_start).
"""

from contextlib import ExitStack

import numpy as np

import concourse.bass as bass
import concourse.tile as tile
from concourse import bacc, mybir
from concourse.bass_utils import run_bass_kernel_spmd

F32 = mybir.dt.float32
BF16 = mybir.dt.bfloat16

B, F, N, D = 1, 16, 256, 1024
H, HD = 16, 64
CS = 4
C = F // CS            # 4 chunks
L = CS * N             # 1024 tokens per chunk
T = F * N              # 4096 tokens
NCORES = 8
HPC = H // NCORES      # 2 heads per core
HDB = HPC * HD         # 128 hd dims per core
SCALE = 1.0 / np.sqrt(HD)

NDT = D // 128         # 8 contraction tiles for projections
NQT = T // 512         # 8 query-projection tiles (512 tokens each)
NCT = T // 128         # 32 ctx tiles of 128 tokens
NTB = T // 128         # 32 token tiles (output side)

# ctx tiles seen by chunk c (128-token tiles, global index)
CTS = [list(range(max(0, 8 * (c - 1)), 8 * (c + 1))) for c in range(C)]
PAIRS = [[(t[i], t[i + 1]) for i in range(0, len(t), 2)] for t in CTS]
# attention pair events: (c, th, h, p); th = 512-query half of the chunk
EVENTS = [
    (c, th, h, p)
    for c in range(C)
    for th in (0, 1)
    for h in range(HPC)
    for p in range(len(PAIRS[c]))
]
assert len(EVENTS) == 112

# static PE filler schedule: event index -> list of (kind, idx)
# kinds: Q/K = 512-token q/k projection chain j, V = 128-token v
# projection ct, P = post work for token tile tb (transpose + out-proj)
FILLER = {
    0: [("V", 0), ("V", 1), ("V", 2)],
    1: [("K", 1), ("V", 3)],
    2: [("V", 4), ("V", 5)],
    3: [("V", 6), ("V", 7)],
    4: [("Q", 1)],
    5: [("K", 2)],
    6: [("Q", 2)],
    7: [("K", 3)],
    8: [("Q", 3)],
    9: [("V", 8)], 10: [("V", 9)], 11: [("V", 10)], 12: [("V", 11)],
    13: [("P", 0)], 14: [("P", 1)], 15: [("P", 2)],
    16: [("V", 12)], 17: [("V", 13)], 18: [("V", 14)], 19: [("V", 15)],
    20: [("P", 3)], 21: [("P", 4)], 22: [("P", 5)], 23: [("P", 6)],
    24: [("P", 7)],
    25: [("Q", 4)], 26: [("K", 4)], 27: [("Q", 5)], 28: [("K", 5)],
    33: [("P", 8)], 34: [("P", 9)], 35: [("P", 10)], 36: [("P", 11)],
    37: [("V", 16)], 38: [("V", 17)], 39: [("V", 18)], 40: [("V", 19)],
    48: [("V", 20)], 49: [("V", 21)], 50: [("V", 22)], 51: [("V", 23)],
    52: [("P", 12)], 53: [("P", 13)], 54: [("P", 14)], 55: [("P", 15)],
    56: [("Q", 6)], 57: [("K", 6)], 58: [("Q", 7)], 59: [("K", 7)],
    65: [("P", 16)], 66: [("P", 17)], 67: [("P", 18)], 68: [("P", 19)],
    69: [("V", 24)], 70: [("V", 25)], 71: [("V", 26)], 72: [("V", 27)],
    80: [("V", 28)], 81: [("V", 29)], 82: [("V", 30)], 83: [("V", 31)],
    84: [("P", 20)], 85: [("P", 21)], 86: [("P", 22)], 87: [("P", 23)],
    97: [("P", 24)], 98: [("P", 25)], 99: [("P", 26)], 100: [("P", 27)],
}


def build_kernel(nc, tc, outs, ins, ctx):
    xt, wq, wk, wv, wo, ident = (
        ins["xt"], ins["wq"], ins["wk"], ins["wv"], ins["wo"], ins["ident"],
    )
    y = outs["y"]

    # ---- SBUF pools ----
    wpool = ctx.enter_context(tc.tile_pool(name="weights", bufs=1))
    xpool = ctx.enter_context(tc.tile_pool(name="xtiles", bufs=1))
    qk_pool = ctx.enter_context(tc.tile_pool(name="qk", bufs=1))
    v_pool = ctx.enter_context(tc.tile_pool(name="vsb", bufs=1))
    a_pool = ctx.enter_context(tc.tile_pool(name="attn", bufs=4))
    osb_pool = ctx.enter_context(tc.tile_pool(name="osb", bufs=12))
    r_pool = ctx.enter_context(tc.tile_pool(name="recip", bufs=4))
    ot_pool = ctx.enter_context(tc.tile_pool(name="ot", bufs=1))
    ysb_pool = ctx.enter_context(tc.tile_pool(name="ysb", bufs=2))
    # PSUM: s 2x2 banks + o 1 + p 1 + y 2 = 8 banks exactly
    pp = ctx.enter_context(tc.tile_pool(name="pp", bufs=1, space="PSUM"))

    # persistent SBUF tiles
    wq_sb = wpool.tile([128, D], BF16, name="wqsb")
    wk_sb = wpool.tile([128, D], BF16, name="wksb")
    wv_sb = wpool.tile([128, D], BF16, name="wvsb")
    wo_sb = wpool.tile([128, D], BF16, name="wosb")
    id_sb = wpool.tile([128, 128], BF16, name="idsb")
    xt_t = [xpool.tile([128, T], BF16, name=f"xt{d}") for d in range(NDT)]
    qt_sb = qk_pool.tile([128, T], BF16, name="qtsb")   # [2 heads x 64, T]
    kt_sb = qk_pool.tile([128, T], BF16, name="ktsb")
    # V per head: [128 ctx-token partitions, 32 ctx tiles, 64 hd + ones col]
    v_sb = [v_pool.tile([128, NCT, HD + 1], BF16, name=f"vsb{h}")
            for h in range(HPC)]
    ot_sb = ot_pool.tile([128, T], BF16, name="otsb")   # O^T normalized

    # ---- input DMAs (HWDGE is serial: few, large transfers; ordered so
    # the first projection chains unblock as early as possible) ----
    nc.sync.dma_start(wq_sb[:], wq[:, :])
    nc.sync.dma_start(wk_sb[:], wk[:, :])
    for d in range(NDT):   # tokens 0..511 of every contraction tile
        nc.sync.dma_start(xt_t[d][:, 0:512], xt[d * 128:(d + 1) * 128, 0:512])
    nc.sync.dma_start(wv_sb[:], wv[:, :])
    nc.sync.dma_start(wo_sb[:], wo[:, :])
    nc.sync.dma_start(id_sb[:], ident[:, :])
    for d in range(NDT):   # tokens 512..1023
        nc.sync.dma_start(xt_t[d][:, 512:1024], xt[d * 128:(d + 1) * 128, 512:1024])
    for d in range(NDT):   # rest
        nc.sync.dma_start(xt_t[d][:, 1024:T], xt[d * 128:(d + 1) * 128, 1024:T])

    # ones columns of V (softmax denominator rides the AV matmul)
    for h in range(HPC):
        nc.vector.memset(v_sb[h][:, :, HD], 1.0)

    # ---- emission helpers ----
    def emit_Q(j, which):
        w_sb, dst = (wq_sb, qt_sb) if which == "Q" else (wk_sb, kt_sb)
        ps = pp.tile([128, 512], F32, tag="p", bufs=1, name=f"{which}ps{j}")
        for d in range(NDT):
            nc.tensor.matmul(
                ps[:], w_sb[:, d * 128:(d + 1) * 128],
                xt_t[d][:, j * 512:(j + 1) * 512],
                start=(d == 0), stop=(d == NDT - 1),
            )
        nc.vector.tensor_copy(dst[:, j * 512:(j + 1) * 512], ps[:])

    def emit_V(ct):
        ps = pp.tile([128, 128], F32, tag="p", bufs=1, name=f"vps{ct}")
        for d in range(NDT):
            nc.tensor.matmul(
                ps[:], xt_t[d][:, ct * 128:(ct + 1) * 128],
                wv_sb[:, d * 128:(d + 1) * 128],
                start=(d == 0), stop=(d == NDT - 1),
            )
        for h in range(HPC):
            nc.vector.tensor_copy(
                v_sb[h][:, ct, 0:HD], ps[:, h * HD:(h + 1) * HD])

    osb_tiles = {}

    def get_osb(tb):
        if tb not in osb_tiles:
            osb_tiles[tb] = osb_pool.tile(
                [128, 128], BF16, tag="osb", name=f"osb{tb}")
        return osb_tiles[tb]

    ysb_tiles = {}

    def emit_post(tb):
        g = tb // 4
        if g not in ysb_tiles:
            ysb_tiles[g] = ysb_pool.tile(
                [128, 4, D], BF16, tag="ysb", name=f"ysb{g}")
        ysb = ysb_tiles[g]
        t_ps = pp.tile([128, 128], BF16, tag="p", bufs=1, name=f"tp{tb}")
        nc.tensor.transpose(t_ps[:], osb_tiles[tb][:], id_sb[:])
        nc.vector.tensor_copy(ot_sb[:, tb * 128:(tb + 1) * 128], t_ps[:])
        for dh in range(D // 512):
            yp = pp.tile([128, 512], F32, tag="y", bufs=2, name=f"yp{tb}_{dh}")
            nc.tensor.matmul(
                yp[:],
                ot_sb[:, tb * 128:(tb + 1) * 128],
                wo_sb[:, dh * 512:(dh + 1) * 512],
                start=True, stop=True,
            )
            nc.vector.tensor_copy(
                ysb[:, tb % 4, dh * 512:(dh + 1) * 512], yp[:])
        if tb % 4 == 3:
            # y is [8, 4, 128, 1024]; dst iterates [p][tb][d] to match the
            # SBUF staging tile's element order
            nc.sync.dma_start(
                y[g:g + 1].transpose([0, 2, 1, 3]), ysb[:])

    def emit_filler(kind, idx):
        if kind in ("Q", "K"):
            emit_Q(idx, kind)
        elif kind == "V":
            emit_V(idx)
        else:
            emit_post(idx)

    def emit_spair(e, c, th, h, p):
        ct0, ct1 = PAIRS[c][p]
        tok0 = c * L + th * 512
        hr = slice(h * HD, (h + 1) * HD)
        s_t = pp.tile([128, 1024], F32, tag="s", bufs=2, name=f"s{e}")
        nc.tensor.matmul(
            s_t[:, 0:512], kt_sb[hr, ct0 * 128:(ct0 + 1) * 128],
            qt_sb[hr, tok0:tok0 + 512], start=True, stop=True)
        nc.tensor.matmul(
            s_t[:, 512:1024], kt_sb[hr, ct1 * 128:(ct1 + 1) * 128],
            qt_sb[hr, tok0:tok0 + 512], start=True, stop=True)
        a_t = a_pool.tile([128, 1024], BF16, tag="a", name=f"a{e}")
        nc.scalar.activation(
            a_t[:], s_t[:], mybir.ActivationFunctionType.Exp, scale=SCALE)
        return a_t

    o_tiles = {}

    def emit_oinit(c, th, h):
        # matmul start=True zeroes the WHOLE psum bank, so the four
        # interleaved qb accumulation regions cannot each use start.
        # Zero the tile once on DVE and accumulate with start=False.
        o_t = pp.tile(
            [128, 4, HD + 1], F32, tag="o", bufs=1, name=f"o{c}_{th}_{h}")
        o_tiles[(c, th, h)] = o_t
        nc.vector.memset(o_t[:], 0.0)

    def emit_av(c, th, h, p, a_t):
        npair = len(PAIRS[c])
        o_t = o_tiles[(c, th, h)]
        for ci, ct in enumerate(PAIRS[c][p]):
            for qb in range(4):
                nc.tensor.matmul(
                    o_t[:, qb, :],
                    a_t[:, ci * 512 + qb * 128: ci * 512 + (qb + 1) * 128],
                    v_sb[h][:, ct, :],
                    start=False,
                    stop=(p == npair - 1 and ci == 1),
                    skip_group_check=True,
                )

    def emit_normalize(c, th, h):
        o_t = o_tiles[(c, th, h)]
        r = r_pool.tile([128, 4], F32, tag="r", name=f"r{c}_{th}_{h}")
        nc.vector.reciprocal(r[:], o_t[:, :, HD])
        for qb in range(4):
            tb = c * 8 + th * 4 + qb
            osb = get_osb(tb)
            nc.vector.tensor_scalar_mul(
                osb[:, h * HD:(h + 1) * HD], o_t[:, qb, 0:HD], r[:, qb:qb + 1])

    # ---- main interleaved emission ----
    emit_Q(0, "Q")
    emit_Q(0, "K")

    pending_av = None   # (c, th, h, p, a_t) awaiting emission (lag 1)
    prev_cthh = None
    for e, (c, th, h, p) in enumerate(EVENTS):
        if prev_cthh is not None and prev_cthh != (c, th, h):
            # drain previous (c, th, h): tail AV + normalization
            pc, pth, ph, pp_, pa = pending_av
            emit_av(pc, pth, ph, pp_, pa)
            pending_av = None
            emit_normalize(pc, pth, ph)
        if p == 0:
            emit_oinit(c, th, h)
        a_t = emit_spair(e, c, th, h, p)
        if pending_av is not None:
            emit_av(*pending_av)
        pending_av = (c, th, h, p, a_t)
        prev_cthh = (c, th, h)
        for kind, idx in FILLER.get(e, ()):
            emit_filler(kind, idx)

    # tail
    pc, pth, ph, pp_, pa = pending_av
    emit_av(pc, pth, ph, pp_, pa)
    emit_normalize(pc, pth, ph)
    for tb in range(28, 32):
        emit_post(tb)


_CACHE = {}


def _build():
    if "nc" in _CACHE:
        return _CACHE["nc"]
    nc = bacc.Bacc(
        "TRN2",
        target_bir_lowering=False,
        debug=False,
        enable_asserts=False,
        num_devices=NCORES,
    )
    ins = {
        "xt": nc.dram_tensor("xt", [D, T], BF16, kind="ExternalInput").ap(),
        "wq": nc.dram_tensor("wq", [128, D], BF16, kind="ExternalInput").ap(),
        "wk": nc.dram_tensor("wk", [128, D], BF16, kind="ExternalInput").ap(),
        "wv": nc.dram_tensor("wv", [128, D], BF16, kind="ExternalInput").ap(),
        "wo": nc.dram_tensor("wo", [128, D], BF16, kind="ExternalInput").ap(),
        "ident": nc.dram_tensor("ident", [128, 128], BF16, kind="ExternalInput").ap(),
    }
    outs = {"y": nc.dram_tensor(
        "y", [T // 512, 4, 128, D], BF16, kind="ExternalOutput").ap()}
    with tile.TileContext(nc, trace_sim=False) as tc:
        with ExitStack() as kctx:
            build_kernel(nc, tc, outs, ins, kctx)
    nc.compile()
    _CACHE["nc"] = nc
    return nc


def make_in_maps(x, Wq, Wk, Wv, Wo, bo):
    BF = mybir.dt.np(mybir.dt.bfloat16)
    xt = np.ascontiguousarray(
        np.asarray(x, dtype=np.float32).reshape(T, D).T
    ).astype(BF)
    ident = np.eye(128, dtype=np.float32).astype(BF)

    def pack(w):  # [1024, 128] -> [128, 8*128], ktile d at cols d*128
        return np.ascontiguousarray(np.concatenate(
            [w[d * 128:(d + 1) * 128, :] for d in range(NDT)], axis=1
        )).astype(BF)

    in_maps = []
    for core in range(NCORES):
        hs = slice(core * HDB, (core + 1) * HDB)
        in_maps.append({
            "xt": xt,
            "wq": pack(np.asarray(Wq, np.float32)[:, hs]),
            "wk": pack(np.asarray(Wk, np.float32)[:, hs]),
            "wv": pack(np.asarray(Wv, np.float32)[:, hs]),
            "wo": np.ascontiguousarray(
                np.asarray(Wo, np.float32)[hs, :]).astype(BF),
            "ident": ident,
        })
    return in_maps


def kernel(x, Wq, Wk, Wv, Wo, bo, _trace=False, _tmpdir=None):
    nc = _build()
    in_maps = make_in_maps(x, Wq, Wk, Wv, Wo, bo)
    res = run_bass_kernel_spmd(
        nc, in_maps, core_ids=list(range(NCORES)),
        trace=_trace, tmpdir=_tmpdir,
        **({"trace_cores": list(range(NCORES))} if _trace else {}),
    )
    if _trace:
        kernel.last_results = res
    y = np.zeros((T, D), dtype=np.float32)
    for r in res.results:
        y += np.asarray(r["y"], dtype=np.float32).reshape(T, D)
    y += np.asarray(bo, dtype=np.float32).reshape(1, D)
    return y.reshape(B, F, N, D)
